# revision 16
# baseline (speedup 1.0000x reference)
"""GRU decoder (nn_Decoder) Trainium2 Bass kernel.

Strategy: pure data parallelism — batch B=8192 sharded over 8 NeuronCores
(1024 rows each), all weights replicated. On-device layout keeps features on
the partition axis and batch on the free axis (h.T is [H, B_c]), so the GRU
recurrence matmuls are stationary-weight PE matmuls streaming the batch.

Per core:
  - 3-layer MLP (fp32 PE matmuls) turns latent.T into the initial hidden
    state h0.T, stored bf16.
  - 65 fully unrolled GRU steps. Input-side gate pre-activations come from a
    one-hot matmul: gi_vocab = embed @ W_ih.T is only [32, 768], and the
    one-hot of the token (built host-side, with an extra constant-1 row that
    folds the input biases into the same matmul as a K=33 contraction) selects
    rows on the PE. Recurrent side is W_hh.T matmuls against h.T (bf16).
    r/z gates: single sigmoid ACT op over a 4-bank PSUM tile (biases already
    folded in). n gate: scalar_tensor_tensor fuses (h_n + b_hh_n) * r, an
    identity matmul accumulates it onto the i_n PSUM bank, tanh reads PSUM.
    h_new = n + z*(h-n) split across GPSIMD and DVE.
  - Per step, the two output projections run on the PE and the result is
    DMA'd straight into the [B_c, T, A] output slice.
"""

import numpy as np
import ml_dtypes

B, L, H, A, T, E = 8192, 128, 256, 32, 65, 8
NCORES = 8
BC = B // NCORES          # 1024 batch rows per core
NCH = 2                   # batch chunks per step (free dim 512 each)
CH = BC // NCH            # 512
G3 = 3 * H                # 768

BF16 = ml_dtypes.bfloat16

_CACHE = {}


def _build(trace=False):
    """Build + finalize the bass module. Returns (nc, meta)."""
    import concourse.bass as bass
    import concourse.bacc as bacc
    import concourse.tile as tile
    from concourse import mybir
    from contextlib import ExitStack

    f32 = mybir.dt.float32
    bf16 = mybir.dt.bfloat16
    Alu = mybir.AluOpType
    Act = mybir.ActivationFunctionType

    nc = bacc.Bacc("TRN2", target_bir_lowering=False, debug=False)

    lat = nc.dram_tensor("lat", [128, BC], f32, kind="ExternalInput")
    oh = nc.dram_tensor("oh", [T, A + 1, BC], bf16, kind="ExternalInput")
    whh = nc.dram_tensor("whh", [128, 2, G3], bf16, kind="ExternalInput")
    giv = nc.dram_tensor("giv", [A + 1, G3], bf16, kind="ExternalInput")
    wd0 = nc.dram_tensor("wd0", [128, H], f32, kind="ExternalInput")
    wd1 = nc.dram_tensor("wd1", [128, 2, H], f32, kind="ExternalInput")
    wd2 = nc.dram_tensor("wd2", [128, 2, H], f32, kind="ExternalInput")
    wp1 = nc.dram_tensor("wp1", [128, 2, A], bf16, kind="ExternalInput")
    wp2 = nc.dram_tensor("wp2", [128, A], bf16, kind="ExternalInput")
    bias = nc.dram_tensor("bias", [128, 9], f32, kind="ExternalInput")
    bp2b = nc.dram_tensor("bp2b", [1, 4 * A], f32, kind="ExternalInput")
    ident = nc.dram_tensor("ident", [128, 128], bf16, kind="ExternalInput")
    out = nc.dram_tensor("out", [BC, T, A], f32, kind="ExternalOutput")

    outv = out.rearrange("(c j p) t a -> c p j t a", c=NCH, j=4, p=128)

    with ExitStack() as ctx:
        tc = ctx.enter_context(tile.TileContext(nc))
        const = ctx.enter_context(tc.tile_pool(name="const", bufs=1))
        hp = ctx.enter_context(tc.tile_pool(name="hp", bufs=3))
        work = ctx.enter_context(tc.tile_pool(name="work", bufs=3))
        ohp = ctx.enter_context(tc.tile_pool(name="ohp", bufs=4))
        outp = ctx.enter_context(tc.tile_pool(name="outp", bufs=4))
        psum = ctx.enter_context(tc.tile_pool(name="psum", bufs=1, space="PSUM"))

        # ---- load constants ----
        lat_sb = const.tile([128, BC], f32, tag="lat")
        nc.sync.dma_start(out=lat_sb[:], in_=lat[:])
        whh_sb = const.tile([128, 2, G3], bf16, tag="whh")
        nc.sync.dma_start(out=whh_sb[:], in_=whh[:])
        giv_sb = const.tile([A + 1, G3], bf16, tag="giv")
        nc.sync.dma_start(out=giv_sb[:], in_=giv[:])
        wd0_sb = const.tile([128, H], f32, tag="wd0")
        nc.sync.dma_start(out=wd0_sb[:], in_=wd0[:])
        wd1_sb = const.tile([128, 2, H], f32, tag="wd1")
        nc.sync.dma_start(out=wd1_sb[:], in_=wd1[:])
        wd2_sb = const.tile([128, 2, H], f32, tag="wd2")
        nc.sync.dma_start(out=wd2_sb[:], in_=wd2[:])
        wp1_sb = const.tile([128, 2, A], bf16, tag="wp1")
        nc.sync.dma_start(out=wp1_sb[:], in_=wp1[:])
        wp2_sb = const.tile([128, A], bf16, tag="wp2")
        nc.sync.dma_start(out=wp2_sb[:], in_=wp2[:])
        bias_sb = const.tile([128, 9], f32, tag="bias")
        nc.sync.dma_start(out=bias_sb[:], in_=bias[:])
        bp2_sb = const.tile([128, 4 * A], f32, tag="bp2")
        nc.sync.dma_start(
            out=bp2_sb[:],
            in_=bass.AP(tensor=bp2b, offset=0, ap=[[0, 128], [1, 4 * A]]),
        )
        id_sb = const.tile([128, 128], bf16, tag="ident")
        nc.sync.dma_start(out=id_sb[:], in_=ident[:])

        # ---- MLP prologue: h0 = (relu(relu(lat@Wd0+b)@Wd1+b))@Wd2+b ----
        h1 = [work.tile([128, BC], f32, tag=f"mlp{m}", name=f"mlp{m}") for m in range(2)]
        for c in range(NCH):
            cs = slice(c * CH, (c + 1) * CH)
            ps = psum.tile([128, 2, CH], f32, tag="pr")
            for m in range(2):
                nc.tensor.matmul(
                    ps[:, m, :], wd0_sb[:, m * 128:(m + 1) * 128], lat_sb[:, cs],
                    start=True, stop=True,
                )
            for m in range(2):
                nc.vector.tensor_scalar(
                    out=h1[m][:, cs], in0=ps[:, m, :],
                    scalar1=bias_sb[:, 3:4] if m == 0 else bias_sb[:, 4:5],
                    scalar2=0.0, op0=Alu.add, op1=Alu.max,
                )
        h2 = [work.tile([128, BC], f32, tag=f"mlp2{m}", name=f"mlp2{m}") for m in range(2)]
        for c in range(NCH):
            cs = slice(c * CH, (c + 1) * CH)
            ps = psum.tile([128, 2, CH], f32, tag="phn")
            for m in range(2):
                for kc in range(2):
                    nc.tensor.matmul(
                        ps[:, m, :], wd1_sb[:, kc, m * 128:(m + 1) * 128],
                        h1[kc][:, cs], start=(kc == 0), stop=(kc == 1),
                    )
            for m in range(2):
                nc.vector.tensor_scalar(
                    out=h2[m][:, cs], in0=ps[:, m, :],
                    scalar1=bias_sb[:, 5:6] if m == 0 else bias_sb[:, 6:7],
                    scalar2=0.0, op0=Alu.add, op1=Alu.max,
                )
        h_cur = [hp.tile([128, BC], bf16, tag=f"h{m}", name=f"h0_{m}") for m in range(2)]
        for c in range(NCH):
            cs = slice(c * CH, (c + 1) * CH)
            ps = psum.tile([128, 2, CH], f32, tag="pin")
            for m in range(2):
                for kc in range(2):
                    nc.tensor.matmul(
                        ps[:, m, :], wd2_sb[:, kc, m * 128:(m + 1) * 128],
                        h2[kc][:, cs], start=(kc == 0), stop=(kc == 1),
                    )
            for m in range(2):
                nc.vector.tensor_scalar_add(
                    out=h_cur[m][:, cs], in0=ps[:, m, :],
                    scalar1=bias_sb[:, 7:8] if m == 0 else bias_sb[:, 8:9],
                )

        # ---- GRU steps ----
        PACKED_PROJ = False

        def emit_proj(h_tiles, tp):
            """Output projections for step tp (pipelined one step behind)."""
            for c in range(NCH):
                if PACKED_PROJ:
                    # p1 = relu(h @ Wp1 + bp1), packed: batch-subtile j ->
                    # col group j, so p1t is [4*32, 128]
                    p1ps = psum.tile([128, 128], f32, tag="phn",
                                     name=f"p1ps_{tp}_{c}")
                    for j in range(4):
                        bs = slice(c * CH + j * 128, c * CH + (j + 1) * 128)
                        for kc in range(2):
                            nc.tensor.matmul(
                                p1ps[32 * j:32 * (j + 1), :], wp1_sb[:, kc, :],
                                h_tiles[kc][:, bs],
                                start=(kc == 0), stop=(kc == 1),
                                tile_position=(0, 32 * j),
                            )
                    p1t = work.tile([128, 128], bf16, tag="p1t",
                                    name=f"p1t_{tp}_{c}")
                    nc.scalar.activation(
                        out=p1t[:], in_=p1ps[:], func=Act.Relu,
                        bias=bias_sb[:, 2:3],
                    )
                    # p2 = p1 @ Wp2 + bp2, row-group packed; out [128, 4*32]
                    p2ps = psum.tile([128, 4 * A], f32, tag="pin",
                                     name=f"p2ps_{tp}_{c}")
                    for j in range(4):
                        nc.tensor.matmul(
                            p2ps[:, j * A:(j + 1) * A],
                            p1t[32 * j:32 * (j + 1), :],
                            wp2_sb[32 * j:32 * (j + 1), :],
                            start=True, stop=True, tile_position=(32 * j, 0),
                        )
                else:
                    # simple: p1.T [32, 512] per chunk, p2 via 4 plain MMs
                    p1ps = psum.tile([A, CH], f32, tag="phn",
                                     name=f"p1ps_{tp}_{c}")
                    for kc in range(2):
                        nc.tensor.matmul(
                            p1ps[:], wp1_sb[:, kc, :],
                            h_tiles[kc][:, c * CH:(c + 1) * CH],
                            start=(kc == 0), stop=(kc == 1),
                        )
                    p1t = work.tile([A, CH], bf16, tag="p1t",
                                    name=f"p1t_{tp}_{c}")
                    nc.scalar.activation(
                        out=p1t[:], in_=p1ps[:], func=Act.Relu,
                        bias=bias_sb[0:A, 2:3],
                    )
                    p2ps = psum.tile([128, 4 * A], f32, tag="pin",
                                     name=f"p2ps_{tp}_{c}")
                    for j in range(4):
                        nc.tensor.matmul(
                            p2ps[:, j * A:(j + 1) * A],
                            p1t[:, j * 128:(j + 1) * 128], wp2_sb[0:A, :],
                            start=True, stop=True,
                        )
                outsb = outp.tile([128, 4 * A], f32, tag="outsb",
                                  name=f"outsb_{tp}_{c}")
                nc.vector.tensor_add(outsb[:], p2ps[:], bp2_sb[:])
                nc.sync.dma_start(
                    out=outv[c][:, :, tp, :],
                    in_=outsb.rearrange("p (j a) -> p j a", j=4),
                )

        h_prev = None
        for t in range(T):
            oh_t = ohp.tile([A + 1, BC], bf16, tag="oh", name=f"oh_{t}")
            nc.sync.dma_start(out=oh_t[:], in_=oh[t])

            h_new = [hp.tile([128, BC], bf16, tag=f"h{m}", name=f"h_{t}_{m}")
                     for m in range(2)]
            for c in range(NCH):
                cs = slice(c * CH, (c + 1) * CH)
                pr = psum.tile([128, 2, CH], f32, tag="pr", name=f"pr_{t}_{c}")
                pz = psum.tile([128, 2, CH], f32, tag="pz", name=f"pz_{t}_{c}")
                phn = psum.tile([128, 2, CH], f32, tag="phn", name=f"phn_{t}_{c}")
                pin = psum.tile([128, 2, CH], f32, tag="pin", name=f"pin_{t}_{c}")

                for m in range(2):
                    nc.tensor.matmul(
                        pin[:, m, :], giv_sb[:, 512 + m * 128:512 + (m + 1) * 128],
                        oh_t[:, cs], start=True, stop=False,
                    )
                # one-hot matmuls first: they are always ready, so they
                # prefill the PSUM groups while PE waits for h_new
                for m in range(2):
                    nc.tensor.matmul(
                        pr[:, m, :], giv_sb[:, m * 128:(m + 1) * 128],
                        oh_t[:, cs], start=True, stop=False,
                    )
                for m in range(2):
                    nc.tensor.matmul(
                        pz[:, m, :], giv_sb[:, 256 + m * 128:256 + (m + 1) * 128],
                        oh_t[:, cs], start=True, stop=False,
                    )
                # recurrent matmuls: kc=0 first (h row-tile 0 is ready first),
                # chain-feeding targets (phn, pr) before pz
                for kc in range(2):
                    for m in range(2):
                        nc.tensor.matmul(
                            phn[:, m, :],
                            whh_sb[:, kc, 512 + m * 128:512 + (m + 1) * 128],
                            h_cur[kc][:, cs], start=(kc == 0), stop=(kc == 1),
                        )
                    for m in range(2):
                        nc.tensor.matmul(
                            pr[:, m, :], whh_sb[:, kc, m * 128:(m + 1) * 128],
                            h_cur[kc][:, cs], start=False, stop=(kc == 1),
                        )
                    for m in range(2):
                        nc.tensor.matmul(
                            pz[:, m, :],
                            whh_sb[:, kc, 256 + m * 128:256 + (m + 1) * 128],
                            h_cur[kc][:, cs], start=False, stop=(kc == 1),
                        )

                # per row-tile fast path: sigmoid(r_m) -> npre -> pin+= ->
                # tanh_m -> combine_m, so m=0 completes without waiting m=1
                rz = work.tile([128, 4, CH], bf16, tag="rz", name=f"rz_{t}_{c}")
                npre = work.tile([128, 2, CH], bf16, tag="npre", name=f"npre_{t}_{c}")
                nsb = work.tile([128, 2, CH], bf16, tag="nsb", name=f"nsb_{t}_{c}")
                t3 = work.tile([128, 2, CH], bf16, tag="t3", name=f"t3_{t}_{c}")
                for m in range(2):
                    nc.scalar.activation(
                        out=rz[:, m, :], in_=pr[:, m, :], func=Act.Sigmoid)
                    nc.vector.scalar_tensor_tensor(
                        out=npre[:, m, :], in0=phn[:, m, :],
                        scalar=bias_sb[:, m:m + 1], in1=rz[:, m, :],
                        op0=Alu.add, op1=Alu.mult,
                    )
                # z = sigmoid(pz) (off the critical chain)
                nc.scalar.activation(out=rz[:, 2:4, :], in_=pz[:], func=Act.Sigmoid)
                for m in range(2):
                    nc.tensor.matmul(
                        pin[:, m, :], id_sb[:], npre[:, m, :],
                        start=False, stop=True,
                    )
                for m in range(2):
                    nc.scalar.activation(
                        out=nsb[:, m, :], in_=pin[:, m, :], func=Act.Tanh)
                    # h_new = n + z*(h - n)
                    nc.vector.tensor_sub(
                        t3[:, m, :], h_cur[m][:, cs], nsb[:, m, :])
                    nc.vector.tensor_mul(t3[:, m, :], rz[:, 2 + m, :], t3[:, m, :])
                    nc.vector.tensor_add(
                        h_new[m][:, cs], nsb[:, m, :], t3[:, m, :])

            if h_prev is not None:
                emit_proj(h_prev, t - 1)
            h_prev = h_new
            h_cur = h_new
        emit_proj(h_prev, T - 1)

    nc.finalize()
    return nc


def _prep_inputs(latent, target, embed, W_ih, b_ih, W_hh, b_hh,
                 Wd0, bd0, Wd1, bd1, Wd2, bd2, Wp1, bp1, Wp2, bp2):
    f32 = np.float32
    latent = np.asarray(latent, dtype=f32)
    target = np.asarray(target)
    embed = np.asarray(embed, dtype=f32)
    W_ih = np.asarray(W_ih, dtype=f32)
    b_ih = np.asarray(b_ih, dtype=f32)
    W_hh = np.asarray(W_hh, dtype=f32)
    b_hh = np.asarray(b_hh, dtype=f32)

    # one-hot tokens (teacher forcing shift), time-major, plus a const-1 row
    tokens = np.concatenate(
        [np.zeros((B, 1), dtype=np.int64), np.asarray(target[:, :-1], dtype=np.int64)],
        axis=1,
    )  # [B, T]
    ohf = np.zeros((T, A + 1, B), dtype=BF16)
    tok_tm = tokens.T  # [T, B]
    for a in range(A):
        ohf[:, a, :] = (tok_tm == a)
    ohf[:, A, :] = 1.0

    giv = embed @ W_ih.T  # [A, 3H]
    brow = np.empty((G3,), dtype=f32)
    brow[: 2 * H] = (b_ih + b_hh)[: 2 * H]
    brow[2 * H:] = b_ih[2 * H:]
    giv_aug = np.concatenate([giv, brow[None, :]], axis=0).astype(BF16)  # [33, 768]

    whhT = np.ascontiguousarray(W_hh.T)  # [H, 3H]
    whh_l = np.ascontiguousarray(
        whhT.reshape(2, 128, G3).transpose(1, 0, 2)
    ).astype(BF16)  # [128, 2, 768]

    wd0_l = np.ascontiguousarray(np.asarray(Wd0, dtype=f32))          # [128, 256]
    wd1_l = np.ascontiguousarray(
        np.asarray(Wd1, dtype=f32).reshape(2, 128, H).transpose(1, 0, 2))
    wd2_l = np.ascontiguousarray(
        np.asarray(Wd2, dtype=f32).reshape(2, 128, H).transpose(1, 0, 2))
    wp1_l = np.ascontiguousarray(
        np.asarray(Wp1, dtype=f32).reshape(2, 128, A).transpose(1, 0, 2)).astype(BF16)
    wp2_l = np.ascontiguousarray(
        np.tile(np.asarray(Wp2, dtype=f32), (4, 1))).astype(BF16)  # [128, 32]

    bias_pack = np.zeros((128, 9), dtype=f32)
    bias_pack[:, 0] = b_hh[2 * H: 2 * H + 128]
    bias_pack[:, 1] = b_hh[2 * H + 128:]
    bias_pack[:, 2] = np.tile(np.asarray(bp1, dtype=f32), 4)
    bias_pack[:, 3] = np.asarray(bd0, dtype=f32)[:128]
    bias_pack[:, 4] = np.asarray(bd0, dtype=f32)[128:]
    bias_pack[:, 5] = np.asarray(bd1, dtype=f32)[:128]
    bias_pack[:, 6] = np.asarray(bd1, dtype=f32)[128:]
    bias_pack[:, 7] = np.asarray(bd2, dtype=f32)[:128]
    bias_pack[:, 8] = np.asarray(bd2, dtype=f32)[128:]

    bp2b = np.ascontiguousarray(
        np.tile(np.asarray(bp2, dtype=f32), 4)[None, :])  # [1, 128]
    ident = np.eye(128, dtype=BF16)

    latT = np.ascontiguousarray(latent.T)  # [128, B]

    shared = dict(whh=whh_l, giv=giv_aug, wd0=wd0_l, wd1=wd1_l, wd2=wd2_l,
                  wp1=wp1_l, wp2=wp2_l, bias=bias_pack, bp2b=bp2b, ident=ident)
    in_maps = []
    for c in range(NCORES):
        bs = slice(c * BC, (c + 1) * BC)
        m = dict(shared)
        m["lat"] = np.ascontiguousarray(latT[:, bs])
        m["oh"] = np.ascontiguousarray(ohf[:, :, bs])
        in_maps.append(m)
    return in_maps


def kernel(**inputs):
    from concourse.bass_utils import run_bass_kernel_spmd

    if "nc" not in _CACHE:
        _CACHE["nc"] = _build()
    nc = _CACHE["nc"]

    in_maps = _prep_inputs(**inputs)
    res = run_bass_kernel_spmd(nc, in_maps, core_ids=list(range(NCORES)))
    outs = [r["out"] for r in res.results]
    return np.concatenate(outs, axis=0).astype(np.float32)


# revision 19
# speedup vs baseline: 1.0001x; 1.0001x over previous
"""GRU decoder (nn_Decoder) Trainium2 Bass kernel.

Strategy: pure data parallelism — batch B=8192 sharded over 8 NeuronCores
(1024 rows each), all weights replicated. On-device layout keeps features on
the partition axis and batch on the free axis (h.T is [H, B_c]), so the GRU
recurrence matmuls are stationary-weight PE matmuls streaming the batch.

Per core:
  - 3-layer MLP (fp32 PE matmuls) turns latent.T into the initial hidden
    state h0.T, stored bf16.
  - 65 fully unrolled GRU steps. Input-side gate pre-activations come from a
    one-hot matmul: gi_vocab = embed @ W_ih.T is only [32, 768], and the
    one-hot of the token (built host-side, with an extra constant-1 row that
    folds the input biases into the same matmul as a K=33 contraction) selects
    rows on the PE. Recurrent side is W_hh.T matmuls against h.T (bf16).
    r/z gates: single sigmoid ACT op over a 4-bank PSUM tile (biases already
    folded in). n gate: scalar_tensor_tensor fuses (h_n + b_hh_n) * r, an
    identity matmul accumulates it onto the i_n PSUM bank, tanh reads PSUM.
    h_new = n + z*(h-n) on the DVE (bf16, 2x mode).
  - Per step, the two output projections run on the PE and the result is
    DMA'd straight into the [B_c, T, A] output slice.
"""

import numpy as np
import ml_dtypes

B, L, H, A, T, E = 8192, 128, 256, 32, 65, 8
NCORES = 8
BC = B // NCORES          # 1024 batch rows per core
NCH = 2                   # batch chunks per step (free dim 512 each)
CH = BC // NCH            # 512
G3 = 3 * H                # 768

BF16 = ml_dtypes.bfloat16

_CACHE = {}


def _build(trace=False):
    """Build + finalize the bass module. Returns (nc, meta)."""
    import concourse.bass as bass
    import concourse.bacc as bacc
    import concourse.tile as tile
    from concourse import mybir
    from contextlib import ExitStack

    f32 = mybir.dt.float32
    bf16 = mybir.dt.bfloat16
    Alu = mybir.AluOpType
    Act = mybir.ActivationFunctionType

    nc = bacc.Bacc("TRN2", target_bir_lowering=False, debug=False)

    lat = nc.dram_tensor("lat", [128, BC], f32, kind="ExternalInput")
    oh = nc.dram_tensor("oh", [T, A + 1, BC], bf16, kind="ExternalInput")
    whh = nc.dram_tensor("whh", [128, 2, G3], bf16, kind="ExternalInput")
    giv = nc.dram_tensor("giv", [A + 1, G3], bf16, kind="ExternalInput")
    wd0 = nc.dram_tensor("wd0", [128, H], f32, kind="ExternalInput")
    wd1 = nc.dram_tensor("wd1", [128, 2, H], f32, kind="ExternalInput")
    wd2 = nc.dram_tensor("wd2", [128, 2, H], f32, kind="ExternalInput")
    wp1 = nc.dram_tensor("wp1", [128, 2, A], bf16, kind="ExternalInput")
    wp2 = nc.dram_tensor("wp2", [128, A], bf16, kind="ExternalInput")
    bias = nc.dram_tensor("bias", [128, 9], f32, kind="ExternalInput")
    bp2b = nc.dram_tensor("bp2b", [1, 4 * A], f32, kind="ExternalInput")
    ident = nc.dram_tensor("ident", [128, 128], bf16, kind="ExternalInput")
    out = nc.dram_tensor("out", [BC, T, A], f32, kind="ExternalOutput")

    outv = out.rearrange("(c j p) t a -> c p j t a", c=NCH, j=4, p=128)

    with ExitStack() as ctx:
        tc = ctx.enter_context(tile.TileContext(nc))
        const = ctx.enter_context(tc.tile_pool(name="const", bufs=1))
        hp = ctx.enter_context(tc.tile_pool(name="hp", bufs=4))
        work = ctx.enter_context(tc.tile_pool(name="work", bufs=4))
        ohp = ctx.enter_context(tc.tile_pool(name="ohp", bufs=4))
        outp = ctx.enter_context(tc.tile_pool(name="outp", bufs=4))
        psum = ctx.enter_context(tc.tile_pool(name="psum", bufs=1, space="PSUM"))

        # ---- load constants ----
        lat_sb = const.tile([128, BC], f32, tag="lat")
        nc.sync.dma_start(out=lat_sb[:], in_=lat[:])
        whh_sb = const.tile([128, 2, G3], bf16, tag="whh")
        nc.sync.dma_start(out=whh_sb[:], in_=whh[:])
        giv_sb = const.tile([A + 1, G3], bf16, tag="giv")
        nc.sync.dma_start(out=giv_sb[:], in_=giv[:])
        wd0_sb = const.tile([128, H], f32, tag="wd0")
        nc.sync.dma_start(out=wd0_sb[:], in_=wd0[:])
        wd1_sb = const.tile([128, 2, H], f32, tag="wd1")
        nc.sync.dma_start(out=wd1_sb[:], in_=wd1[:])
        wd2_sb = const.tile([128, 2, H], f32, tag="wd2")
        nc.sync.dma_start(out=wd2_sb[:], in_=wd2[:])
        wp1_sb = const.tile([128, 2, A], bf16, tag="wp1")
        nc.sync.dma_start(out=wp1_sb[:], in_=wp1[:])
        wp2_sb = const.tile([128, A], bf16, tag="wp2")
        nc.sync.dma_start(out=wp2_sb[:], in_=wp2[:])
        bias_sb = const.tile([128, 9], f32, tag="bias")
        nc.sync.dma_start(out=bias_sb[:], in_=bias[:])
        bp2_sb = const.tile([128, 4 * A], f32, tag="bp2")
        nc.sync.dma_start(
            out=bp2_sb[:],
            in_=bass.AP(tensor=bp2b, offset=0, ap=[[0, 128], [1, 4 * A]]),
        )
        id_sb = const.tile([128, 128], bf16, tag="ident")
        nc.sync.dma_start(out=id_sb[:], in_=ident[:])

        # ---- MLP prologue: h0 = (relu(relu(lat@Wd0+b)@Wd1+b))@Wd2+b ----
        h1 = [work.tile([128, BC], f32, tag=f"mlp{m}", name=f"mlp{m}") for m in range(2)]
        for c in range(NCH):
            cs = slice(c * CH, (c + 1) * CH)
            ps = psum.tile([128, 2, CH], f32, tag="pr")
            for m in range(2):
                nc.tensor.matmul(
                    ps[:, m, :], wd0_sb[:, m * 128:(m + 1) * 128], lat_sb[:, cs],
                    start=True, stop=True,
                )
            for m in range(2):
                nc.vector.tensor_scalar(
                    out=h1[m][:, cs], in0=ps[:, m, :],
                    scalar1=bias_sb[:, 3:4] if m == 0 else bias_sb[:, 4:5],
                    scalar2=0.0, op0=Alu.add, op1=Alu.max,
                )
        h2 = [work.tile([128, BC], f32, tag=f"mlp2{m}", name=f"mlp2{m}") for m in range(2)]
        for c in range(NCH):
            cs = slice(c * CH, (c + 1) * CH)
            ps = psum.tile([128, 2, CH], f32, tag="phn")
            for m in range(2):
                for kc in range(2):
                    nc.tensor.matmul(
                        ps[:, m, :], wd1_sb[:, kc, m * 128:(m + 1) * 128],
                        h1[kc][:, cs], start=(kc == 0), stop=(kc == 1),
                    )
            for m in range(2):
                nc.vector.tensor_scalar(
                    out=h2[m][:, cs], in0=ps[:, m, :],
                    scalar1=bias_sb[:, 5:6] if m == 0 else bias_sb[:, 6:7],
                    scalar2=0.0, op0=Alu.add, op1=Alu.max,
                )
        h_cur = [hp.tile([128, BC], bf16, tag=f"h{m}", name=f"h0_{m}") for m in range(2)]
        for c in range(NCH):
            cs = slice(c * CH, (c + 1) * CH)
            ps = psum.tile([128, 2, CH], f32, tag="pin")
            for m in range(2):
                for kc in range(2):
                    nc.tensor.matmul(
                        ps[:, m, :], wd2_sb[:, kc, m * 128:(m + 1) * 128],
                        h2[kc][:, cs], start=(kc == 0), stop=(kc == 1),
                    )
            for m in range(2):
                nc.vector.tensor_scalar_add(
                    out=h_cur[m][:, cs], in0=ps[:, m, :],
                    scalar1=bias_sb[:, 7:8] if m == 0 else bias_sb[:, 8:9],
                )

        # ---- GRU steps ----
        PACKED_PROJ = False

        def emit_proj(h_tiles, tp):
            """Output projections for step tp (pipelined one step behind)."""
            for c in range(NCH):
                if PACKED_PROJ:
                    # p1 = relu(h @ Wp1 + bp1), packed: batch-subtile j ->
                    # col group j, so p1t is [4*32, 128]
                    p1ps = psum.tile([128, 128], f32, tag="phn",
                                     name=f"p1ps_{tp}_{c}")
                    for j in range(4):
                        bs = slice(c * CH + j * 128, c * CH + (j + 1) * 128)
                        for kc in range(2):
                            nc.tensor.matmul(
                                p1ps[32 * j:32 * (j + 1), :], wp1_sb[:, kc, :],
                                h_tiles[kc][:, bs],
                                start=(kc == 0), stop=(kc == 1),
                                tile_position=(0, 32 * j),
                            )
                    p1t = work.tile([128, 128], bf16, tag="p1t",
                                    name=f"p1t_{tp}_{c}")
                    nc.scalar.activation(
                        out=p1t[:], in_=p1ps[:], func=Act.Relu,
                        bias=bias_sb[:, 2:3],
                    )
                    # p2 = p1 @ Wp2 + bp2, row-group packed; out [128, 4*32]
                    p2ps = psum.tile([128, 4 * A], f32, tag="pin",
                                     name=f"p2ps_{tp}_{c}")
                    for j in range(4):
                        nc.tensor.matmul(
                            p2ps[:, j * A:(j + 1) * A],
                            p1t[32 * j:32 * (j + 1), :],
                            wp2_sb[32 * j:32 * (j + 1), :],
                            start=True, stop=True, tile_position=(32 * j, 0),
                        )
                else:
                    # simple: p1.T [32, 512] per chunk, p2 via 4 plain MMs
                    p1ps = psum.tile([A, CH], f32, tag="phn",
                                     name=f"p1ps_{tp}_{c}")
                    for kc in range(2):
                        nc.tensor.matmul(
                            p1ps[:], wp1_sb[:, kc, :],
                            h_tiles[kc][:, c * CH:(c + 1) * CH],
                            start=(kc == 0), stop=(kc == 1),
                        )
                    p1t = work.tile([A, CH], bf16, tag="p1t",
                                    name=f"p1t_{tp}_{c}")
                    nc.scalar.activation(
                        out=p1t[:], in_=p1ps[:], func=Act.Relu,
                        bias=bias_sb[0:A, 2:3],
                    )
                    p2ps = psum.tile([128, 4 * A], f32, tag="pin",
                                     name=f"p2ps_{tp}_{c}")
                    for j in range(4):
                        nc.tensor.matmul(
                            p2ps[:, j * A:(j + 1) * A],
                            p1t[:, j * 128:(j + 1) * 128], wp2_sb[0:A, :],
                            start=True, stop=True,
                        )
                outsb = outp.tile([128, 4 * A], f32, tag="outsb",
                                  name=f"outsb_{tp}_{c}")
                nc.vector.tensor_add(outsb[:], p2ps[:], bp2_sb[:])
                nc.sync.dma_start(
                    out=outv[c][:, :, tp, :],
                    in_=outsb.rearrange("p (j a) -> p j a", j=4),
                )

        h_prev = None
        for t in range(T):
            oh_t = ohp.tile([A + 1, BC], bf16, tag="oh", name=f"oh_{t}")
            nc.sync.dma_start(out=oh_t[:], in_=oh[t])

            h_new = [hp.tile([128, BC], bf16, tag=f"h{m}", name=f"h_{t}_{m}")
                     for m in range(2)]
            for c in range(NCH):
                cs = slice(c * CH, (c + 1) * CH)
                pr = psum.tile([128, 2, CH], f32, tag="pr", name=f"pr_{t}_{c}")
                pz = psum.tile([128, 2, CH], f32, tag="pz", name=f"pz_{t}_{c}")
                phn = psum.tile([128, 2, CH], f32, tag="phn", name=f"phn_{t}_{c}")
                pin = psum.tile([128, 2, CH], f32, tag="pin", name=f"pin_{t}_{c}")

                for m in range(2):
                    nc.tensor.matmul(
                        pin[:, m, :], giv_sb[:, 512 + m * 128:512 + (m + 1) * 128],
                        oh_t[:, cs], start=True, stop=False,
                    )
                # one-hot matmuls first: they are always ready, so they
                # prefill the PSUM groups while PE waits for h_new
                for m in range(2):
                    nc.tensor.matmul(
                        pr[:, m, :], giv_sb[:, m * 128:(m + 1) * 128],
                        oh_t[:, cs], start=True, stop=False,
                    )
                for m in range(2):
                    nc.tensor.matmul(
                        pz[:, m, :], giv_sb[:, 256 + m * 128:256 + (m + 1) * 128],
                        oh_t[:, cs], start=True, stop=False,
                    )
                # recurrent matmuls: kc=0 first (h row-tile 0 is ready first),
                # chain-feeding targets (phn, pr) before pz
                for kc in range(2):
                    for m in range(2):
                        nc.tensor.matmul(
                            phn[:, m, :],
                            whh_sb[:, kc, 512 + m * 128:512 + (m + 1) * 128],
                            h_cur[kc][:, cs], start=(kc == 0), stop=(kc == 1),
                        )
                    for m in range(2):
                        nc.tensor.matmul(
                            pr[:, m, :], whh_sb[:, kc, m * 128:(m + 1) * 128],
                            h_cur[kc][:, cs], start=False, stop=(kc == 1),
                        )
                    for m in range(2):
                        nc.tensor.matmul(
                            pz[:, m, :],
                            whh_sb[:, kc, 256 + m * 128:256 + (m + 1) * 128],
                            h_cur[kc][:, cs], start=False, stop=(kc == 1),
                        )

                # per row-tile fast path: sigmoid(r_m) -> npre -> pin+= ->
                # tanh_m -> combine_m, so m=0 completes without waiting m=1
                rz = work.tile([128, 4, CH], bf16, tag="rz", name=f"rz_{t}_{c}")
                npre = work.tile([128, 2, CH], bf16, tag="npre", name=f"npre_{t}_{c}")
                nsb = work.tile([128, 2, CH], bf16, tag="nsb", name=f"nsb_{t}_{c}")
                t3 = work.tile([128, 2, CH], bf16, tag="t3", name=f"t3_{t}_{c}")
                for m in range(2):
                    nc.scalar.activation(
                        out=rz[:, m, :], in_=pr[:, m, :], func=Act.Sigmoid)
                    nc.vector.scalar_tensor_tensor(
                        out=npre[:, m, :], in0=phn[:, m, :],
                        scalar=bias_sb[:, m:m + 1], in1=rz[:, m, :],
                        op0=Alu.add, op1=Alu.mult,
                    )
                # z = sigmoid(pz) (off the critical chain)
                nc.scalar.activation(out=rz[:, 2:4, :], in_=pz[:], func=Act.Sigmoid)
                for m in range(2):
                    nc.tensor.matmul(
                        pin[:, m, :], id_sb[:], npre[:, m, :],
                        start=False, stop=True,
                    )
                for m in range(2):
                    nc.scalar.activation(
                        out=nsb[:, m, :], in_=pin[:, m, :], func=Act.Tanh)
                    # h_new = n + z*(h - n)
                    nc.vector.tensor_sub(
                        t3[:, m, :], h_cur[m][:, cs], nsb[:, m, :])
                    nc.vector.tensor_mul(t3[:, m, :], rz[:, 2 + m, :], t3[:, m, :])
                    nc.vector.tensor_add(
                        h_new[m][:, cs], nsb[:, m, :], t3[:, m, :])

            if h_prev is not None:
                emit_proj(h_prev, t - 1)
            h_prev = h_new
            h_cur = h_new
        emit_proj(h_prev, T - 1)

    nc.finalize()
    return nc


def _prep_inputs(latent, target, embed, W_ih, b_ih, W_hh, b_hh,
                 Wd0, bd0, Wd1, bd1, Wd2, bd2, Wp1, bp1, Wp2, bp2):
    f32 = np.float32
    latent = np.asarray(latent, dtype=f32)
    target = np.asarray(target)
    embed = np.asarray(embed, dtype=f32)
    W_ih = np.asarray(W_ih, dtype=f32)
    b_ih = np.asarray(b_ih, dtype=f32)
    W_hh = np.asarray(W_hh, dtype=f32)
    b_hh = np.asarray(b_hh, dtype=f32)

    # one-hot tokens (teacher forcing shift), time-major, plus a const-1 row
    tokens = np.concatenate(
        [np.zeros((B, 1), dtype=np.int64), np.asarray(target[:, :-1], dtype=np.int64)],
        axis=1,
    )  # [B, T]
    ohf = np.zeros((T, A + 1, B), dtype=BF16)
    tok_tm = tokens.T  # [T, B]
    for a in range(A):
        ohf[:, a, :] = (tok_tm == a)
    ohf[:, A, :] = 1.0

    giv = embed @ W_ih.T  # [A, 3H]
    brow = np.empty((G3,), dtype=f32)
    brow[: 2 * H] = (b_ih + b_hh)[: 2 * H]
    brow[2 * H:] = b_ih[2 * H:]
    giv_aug = np.concatenate([giv, brow[None, :]], axis=0).astype(BF16)  # [33, 768]

    whhT = np.ascontiguousarray(W_hh.T)  # [H, 3H]
    whh_l = np.ascontiguousarray(
        whhT.reshape(2, 128, G3).transpose(1, 0, 2)
    ).astype(BF16)  # [128, 2, 768]

    wd0_l = np.ascontiguousarray(np.asarray(Wd0, dtype=f32))          # [128, 256]
    wd1_l = np.ascontiguousarray(
        np.asarray(Wd1, dtype=f32).reshape(2, 128, H).transpose(1, 0, 2))
    wd2_l = np.ascontiguousarray(
        np.asarray(Wd2, dtype=f32).reshape(2, 128, H).transpose(1, 0, 2))
    wp1_l = np.ascontiguousarray(
        np.asarray(Wp1, dtype=f32).reshape(2, 128, A).transpose(1, 0, 2)).astype(BF16)
    wp2_l = np.ascontiguousarray(
        np.tile(np.asarray(Wp2, dtype=f32), (4, 1))).astype(BF16)  # [128, 32]

    bias_pack = np.zeros((128, 9), dtype=f32)
    bias_pack[:, 0] = b_hh[2 * H: 2 * H + 128]
    bias_pack[:, 1] = b_hh[2 * H + 128:]
    bias_pack[:, 2] = np.tile(np.asarray(bp1, dtype=f32), 4)
    bias_pack[:, 3] = np.asarray(bd0, dtype=f32)[:128]
    bias_pack[:, 4] = np.asarray(bd0, dtype=f32)[128:]
    bias_pack[:, 5] = np.asarray(bd1, dtype=f32)[:128]
    bias_pack[:, 6] = np.asarray(bd1, dtype=f32)[128:]
    bias_pack[:, 7] = np.asarray(bd2, dtype=f32)[:128]
    bias_pack[:, 8] = np.asarray(bd2, dtype=f32)[128:]

    bp2b = np.ascontiguousarray(
        np.tile(np.asarray(bp2, dtype=f32), 4)[None, :])  # [1, 128]
    ident = np.eye(128, dtype=BF16)

    latT = np.ascontiguousarray(latent.T)  # [128, B]

    shared = dict(whh=whh_l, giv=giv_aug, wd0=wd0_l, wd1=wd1_l, wd2=wd2_l,
                  wp1=wp1_l, wp2=wp2_l, bias=bias_pack, bp2b=bp2b, ident=ident)
    in_maps = []
    for c in range(NCORES):
        bs = slice(c * BC, (c + 1) * BC)
        m = dict(shared)
        m["lat"] = np.ascontiguousarray(latT[:, bs])
        m["oh"] = np.ascontiguousarray(ohf[:, :, bs])
        in_maps.append(m)
    return in_maps


def kernel(**inputs):
    from concourse.bass_utils import run_bass_kernel_spmd

    if "nc" not in _CACHE:
        _CACHE["nc"] = _build()
    nc = _CACHE["nc"]

    in_maps = _prep_inputs(**inputs)
    res = run_bass_kernel_spmd(nc, in_maps, core_ids=list(range(NCORES)))
    outs = [r["out"] for r in res.results]
    return np.concatenate(outs, axis=0).astype(np.float32)


# revision 26
# speedup vs baseline: 1.0321x; 1.0320x over previous
"""GRU decoder (nn_Decoder) Trainium2 Bass kernel.

Strategy: pure data parallelism — batch B=8192 sharded over 8 NeuronCores
(1024 rows each), all weights replicated. On-device layout keeps features on
the partition axis and batch on the free axis (h.T is [H, B_c]), so the GRU
recurrence matmuls are stationary-weight PE matmuls streaming the batch.

Per core:
  - 3-layer MLP (fp32 PE matmuls) turns latent.T into the initial hidden
    state h0.T, stored bf16.
  - 65 fully unrolled GRU steps. Input-side gate pre-activations come from a
    one-hot matmul: gi_vocab = embed @ W_ih.T is only [32, 768], and the
    one-hot of the token (built host-side, with an extra constant-1 row that
    folds the input biases into the same matmul as a K=33 contraction) selects
    rows on the PE. Recurrent side is W_hh.T matmuls against h.T (bf16).
    r/z gates: single sigmoid ACT op over a 4-bank PSUM tile (biases already
    folded in). n gate: scalar_tensor_tensor fuses (h_n + b_hh_n) * r, an
    identity matmul accumulates it onto the i_n PSUM bank, tanh reads PSUM.
    h_new = n + z*(h-n) on the DVE (bf16, 2x mode).
  - Per step, the two output projections run on the PE and the result is
    DMA'd straight into the [B_c, T, A] output slice.
"""

import numpy as np
import ml_dtypes

B, L, H, A, T, E = 8192, 128, 256, 32, 65, 8
NCORES = 8
BC = B // NCORES          # 1024 batch rows per core
NCH = 2                   # batch chunks per step (free dim 512 each)
CH = BC // NCH            # 512
G3 = 3 * H                # 768

BF16 = ml_dtypes.bfloat16

_CACHE = {}


def _build(trace=False):
    """Build + finalize the bass module. Returns (nc, meta)."""
    import concourse.bass as bass
    import concourse.bacc as bacc
    import concourse.tile as tile
    from concourse import mybir
    from contextlib import ExitStack

    f32 = mybir.dt.float32
    bf16 = mybir.dt.bfloat16
    Alu = mybir.AluOpType
    Act = mybir.ActivationFunctionType

    nc = bacc.Bacc("TRN2", target_bir_lowering=False, debug=False)

    lat = nc.dram_tensor("lat", [128, BC], f32, kind="ExternalInput")
    oh = nc.dram_tensor("oh", [T, A + 1, BC], bf16, kind="ExternalInput")
    whh = nc.dram_tensor("whh", [128, 2, G3], bf16, kind="ExternalInput")
    giv = nc.dram_tensor("giv", [A + 1, G3], bf16, kind="ExternalInput")
    wd0 = nc.dram_tensor("wd0", [128, H], f32, kind="ExternalInput")
    wd1 = nc.dram_tensor("wd1", [128, 2, H], f32, kind="ExternalInput")
    wd2 = nc.dram_tensor("wd2", [128, 2, H], f32, kind="ExternalInput")
    wp1 = nc.dram_tensor("wp1", [128, 2, A], bf16, kind="ExternalInput")
    wp2 = nc.dram_tensor("wp2", [128, A], bf16, kind="ExternalInput")
    bias = nc.dram_tensor("bias", [128, 9], f32, kind="ExternalInput")
    bp2b = nc.dram_tensor("bp2b", [1, 4 * A], f32, kind="ExternalInput")
    u16 = mybir.dt.uint16
    tokw = nc.dram_tensor("tokw", [128, T, 64], u16, kind="ExternalInput")
    gtab = nc.dram_tensor("gtab", [128, 2, A], bf16, kind="ExternalInput")
    out = nc.dram_tensor("out", [BC, T, A], f32, kind="ExternalOutput")

    outv = out.rearrange("(c j p) t a -> c p j t a", c=NCH, j=4, p=128)

    with ExitStack() as ctx:
        tc = ctx.enter_context(tile.TileContext(nc))
        const = ctx.enter_context(tc.tile_pool(name="const", bufs=1))
        hp = ctx.enter_context(tc.tile_pool(name="hp", bufs=4))
        work = ctx.enter_context(tc.tile_pool(name="work", bufs=4))
        ohp = ctx.enter_context(tc.tile_pool(name="ohp", bufs=4))
        outp = ctx.enter_context(tc.tile_pool(name="outp", bufs=4))
        psum = ctx.enter_context(tc.tile_pool(name="psum", bufs=1, space="PSUM"))

        # ---- load constants ----
        lat_sb = const.tile([128, BC], f32, tag="lat")
        nc.sync.dma_start(out=lat_sb[:], in_=lat[:])
        whh_sb = const.tile([128, 2, G3], bf16, tag="whh")
        nc.sync.dma_start(out=whh_sb[:], in_=whh[:])
        giv_sb = const.tile([A + 1, G3], bf16, tag="giv")
        nc.sync.dma_start(out=giv_sb[:], in_=giv[:])
        wd0_sb = const.tile([128, H], f32, tag="wd0")
        nc.sync.dma_start(out=wd0_sb[:], in_=wd0[:])
        wd1_sb = const.tile([128, 2, H], f32, tag="wd1")
        nc.sync.dma_start(out=wd1_sb[:], in_=wd1[:])
        wd2_sb = const.tile([128, 2, H], f32, tag="wd2")
        nc.sync.dma_start(out=wd2_sb[:], in_=wd2[:])
        wp1_sb = const.tile([128, 2, A], bf16, tag="wp1")
        nc.sync.dma_start(out=wp1_sb[:], in_=wp1[:])
        wp2_sb = const.tile([128, A], bf16, tag="wp2")
        nc.sync.dma_start(out=wp2_sb[:], in_=wp2[:])
        bias_sb = const.tile([128, 9], f32, tag="bias")
        nc.sync.dma_start(out=bias_sb[:], in_=bias[:])
        bp2_sb = const.tile([128, 4 * A], f32, tag="bp2")
        nc.sync.dma_start(
            out=bp2_sb[:],
            in_=bass.AP(tensor=bp2b, offset=0, ap=[[0, 128], [1, 4 * A]]),
        )
        tokw_sb = const.tile([128, T, 64], u16, tag="tokw")
        nc.sync.dma_start(out=tokw_sb[:], in_=tokw[:])
        gtab_sb = const.tile([128, 2, A], bf16, tag="gtab")
        nc.sync.dma_start(out=gtab_sb[:], in_=gtab[:])

        # ---- MLP prologue: h0 = (relu(relu(lat@Wd0+b)@Wd1+b))@Wd2+b ----
        h1 = [work.tile([128, BC], f32, tag=f"mlp{m}", name=f"mlp{m}") for m in range(2)]
        for c in range(NCH):
            cs = slice(c * CH, (c + 1) * CH)
            ps = psum.tile([128, 2, CH], f32, tag="pr", bufs=2)
            for m in range(2):
                nc.tensor.matmul(
                    ps[:, m, :], wd0_sb[:, m * 128:(m + 1) * 128], lat_sb[:, cs],
                    start=True, stop=True,
                )
            for m in range(2):
                nc.vector.tensor_scalar(
                    out=h1[m][:, cs], in0=ps[:, m, :],
                    scalar1=bias_sb[:, 3:4] if m == 0 else bias_sb[:, 4:5],
                    scalar2=0.0, op0=Alu.add, op1=Alu.max,
                )
        h2 = [work.tile([128, BC], f32, tag=f"mlp2{m}", name=f"mlp2{m}") for m in range(2)]
        for c in range(NCH):
            cs = slice(c * CH, (c + 1) * CH)
            ps = psum.tile([128, 2, CH], f32, tag="phn")
            for m in range(2):
                for kc in range(2):
                    nc.tensor.matmul(
                        ps[:, m, :], wd1_sb[:, kc, m * 128:(m + 1) * 128],
                        h1[kc][:, cs], start=(kc == 0), stop=(kc == 1),
                    )
            for m in range(2):
                nc.vector.tensor_scalar(
                    out=h2[m][:, cs], in0=ps[:, m, :],
                    scalar1=bias_sb[:, 5:6] if m == 0 else bias_sb[:, 6:7],
                    scalar2=0.0, op0=Alu.add, op1=Alu.max,
                )
        h_cur = [hp.tile([128, BC], bf16, tag=f"h{m}", name=f"h0_{m}") for m in range(2)]
        for c in range(NCH):
            cs = slice(c * CH, (c + 1) * CH)
            ps = psum.tile([128, 2, CH], f32, tag="pz")
            for m in range(2):
                for kc in range(2):
                    nc.tensor.matmul(
                        ps[:, m, :], wd2_sb[:, kc, m * 128:(m + 1) * 128],
                        h2[kc][:, cs], start=(kc == 0), stop=(kc == 1),
                    )
            for m in range(2):
                nc.vector.tensor_scalar_add(
                    out=h_cur[m][:, cs], in0=ps[:, m, :],
                    scalar1=bias_sb[:, 7:8] if m == 0 else bias_sb[:, 8:9],
                )

        # ---- GRU steps ----
        PACKED_PROJ = False

        def emit_proj(h_tiles, tp):
            """Output projections for step tp (pipelined one step behind)."""
            for c in range(NCH):
                if PACKED_PROJ:
                    # p1 = relu(h @ Wp1 + bp1), packed: batch-subtile j ->
                    # col group j, so p1t is [4*32, 128]
                    p1ps = psum.tile([128, 128], f32, tag="phn",
                                     name=f"p1ps_{tp}_{c}")
                    for j in range(4):
                        bs = slice(c * CH + j * 128, c * CH + (j + 1) * 128)
                        for kc in range(2):
                            nc.tensor.matmul(
                                p1ps[32 * j:32 * (j + 1), :], wp1_sb[:, kc, :],
                                h_tiles[kc][:, bs],
                                start=(kc == 0), stop=(kc == 1),
                                tile_position=(0, 32 * j),
                            )
                    p1t = work.tile([128, 128], bf16, tag="p1t",
                                    name=f"p1t_{tp}_{c}")
                    nc.scalar.activation(
                        out=p1t[:], in_=p1ps[:], func=Act.Relu,
                        bias=bias_sb[:, 2:3],
                    )
                    # p2 = p1 @ Wp2 + bp2, row-group packed; out [128, 4*32]
                    p2ps = psum.tile([128, 4 * A], f32, tag="pz",
                                     name=f"p2ps_{tp}_{c}")
                    for j in range(4):
                        nc.tensor.matmul(
                            p2ps[:, j * A:(j + 1) * A],
                            p1t[32 * j:32 * (j + 1), :],
                            wp2_sb[32 * j:32 * (j + 1), :],
                            start=True, stop=True, tile_position=(32 * j, 0),
                        )
                else:
                    # simple: p1.T [32, 512] per chunk, p2 via 4 plain MMs
                    p1ps = psum.tile([A, CH], f32, tag="phn",
                                     name=f"p1ps_{tp}_{c}")
                    for kc in range(2):
                        nc.tensor.matmul(
                            p1ps[:], wp1_sb[:, kc, :],
                            h_tiles[kc][:, c * CH:(c + 1) * CH],
                            start=(kc == 0), stop=(kc == 1),
                        )
                    p1t = work.tile([A, CH], bf16, tag="p1t",
                                    name=f"p1t_{tp}_{c}")
                    nc.scalar.activation(
                        out=p1t[:], in_=p1ps[:], func=Act.Relu,
                        bias=bias_sb[0:A, 2:3],
                    )
                    p2ps = psum.tile([128, 4 * A], f32, tag="pz",
                                     name=f"p2ps_{tp}_{c}")
                    for j in range(4):
                        nc.tensor.matmul(
                            p2ps[:, j * A:(j + 1) * A],
                            p1t[:, j * 128:(j + 1) * 128], wp2_sb[0:A, :],
                            start=True, stop=True,
                        )
                outsb = outp.tile([128, 4 * A], f32, tag="outsb",
                                  name=f"outsb_{tp}_{c}")
                nc.vector.tensor_add(outsb[:], p2ps[:], bp2_sb[:])
                nc.sync.dma_start(
                    out=outv[c][:, :, tp, :],
                    in_=outsb.rearrange("p (j a) -> p j a", j=4),
                )

        h_prev = None
        for t in range(T):
            oh_t = ohp.tile([A + 1, BC], bf16, tag="oh", name=f"oh_{t}")
            nc.sync.dma_start(out=oh_t[:], in_=oh[t])
            # i_n + b_ih_n gathered by token on the (otherwise idle) GPSIMD
            gin = work.tile([128, 2, BC], bf16, tag="gin", name=f"gin_{t}")
            for m in range(2):
                nc.gpsimd.indirect_copy(
                    out=gin[:, m, :], data=gtab_sb[:, m, :],
                    idxs=tokw_sb[:, t, :],
                    i_know_ap_gather_is_preferred=True,
                )

            h_new = [hp.tile([128, BC], bf16, tag=f"h{m}", name=f"h_{t}_{m}")
                     for m in range(2)]
            for c in range(NCH):
                cs = slice(c * CH, (c + 1) * CH)
                pr = psum.tile([128, 2, CH], f32, tag="pr", bufs=2, name=f"pr_{t}_{c}")
                pz = psum.tile([128, 2, CH], f32, tag="pz", name=f"pz_{t}_{c}")
                phn = psum.tile([128, 2, CH], f32, tag="phn", name=f"phn_{t}_{c}")
                # one-hot matmuls first: they are always ready, so they
                # prefill the PSUM groups while PE waits for h_new
                for m in range(2):
                    nc.tensor.matmul(
                        pr[:, m, :], giv_sb[:, m * 128:(m + 1) * 128],
                        oh_t[:, cs], start=True, stop=False,
                    )
                for m in range(2):
                    nc.tensor.matmul(
                        pz[:, m, :], giv_sb[:, 256 + m * 128:256 + (m + 1) * 128],
                        oh_t[:, cs], start=True, stop=False,
                    )
                # recurrent matmuls: kc=0 first (h row-tile 0 is ready first),
                # chain-feeding targets (phn, pr) before pz
                for kc in range(2):
                    for m in range(2):
                        nc.tensor.matmul(
                            phn[:, m, :],
                            whh_sb[:, kc, 512 + m * 128:512 + (m + 1) * 128],
                            h_cur[kc][:, cs], start=(kc == 0), stop=(kc == 1),
                        )
                    for m in range(2):
                        nc.tensor.matmul(
                            pr[:, m, :], whh_sb[:, kc, m * 128:(m + 1) * 128],
                            h_cur[kc][:, cs], start=False, stop=(kc == 1),
                        )
                    for m in range(2):
                        nc.tensor.matmul(
                            pz[:, m, :],
                            whh_sb[:, kc, 256 + m * 128:256 + (m + 1) * 128],
                            h_cur[kc][:, cs], start=False, stop=(kc == 1),
                        )

                # per row-tile fast path: sigmoid(r_m) -> npre -> pin+= ->
                # tanh_m -> combine_m, so m=0 completes without waiting m=1
                rz = work.tile([128, 4, CH], bf16, tag="rz", name=f"rz_{t}_{c}")
                npre = work.tile([128, 2, CH], bf16, tag="npre", name=f"npre_{t}_{c}")
                t2v = work.tile([128, 2, CH], bf16, tag="t2v", name=f"t2v_{t}_{c}")
                nsb = work.tile([128, 2, CH], bf16, tag="nsb", name=f"nsb_{t}_{c}")
                t3 = work.tile([128, 2, CH], bf16, tag="t3", name=f"t3_{t}_{c}")
                for m in range(2):
                    nc.scalar.activation(
                        out=rz[:, m, :], in_=pr[:, m, :], func=Act.Sigmoid)
                    nc.vector.scalar_tensor_tensor(
                        out=npre[:, m, :], in0=phn[:, m, :],
                        scalar=bias_sb[:, m:m + 1], in1=rz[:, m, :],
                        op0=Alu.add, op1=Alu.mult,
                    )
                    if m == 0:
                        nc.vector.tensor_add(
                            t2v[:, m, :], npre[:, m, :], gin[:, m, cs])
                    else:
                        nc.gpsimd.tensor_add(
                            t2v[:, m, :], npre[:, m, :], gin[:, m, cs])
                # z = sigmoid(pz) (off the critical chain)
                nc.scalar.activation(out=rz[:, 2:4, :], in_=pz[:], func=Act.Sigmoid)
                for m in range(2):
                    nc.scalar.activation(
                        out=nsb[:, m, :], in_=t2v[:, m, :], func=Act.Tanh)
                    # h_new = n + z*(h - n)
                    nc.vector.tensor_sub(
                        t3[:, m, :], h_cur[m][:, cs], nsb[:, m, :])
                    nc.vector.tensor_mul(t3[:, m, :], rz[:, 2 + m, :], t3[:, m, :])
                    nc.vector.tensor_add(
                        h_new[m][:, cs], nsb[:, m, :], t3[:, m, :])

            if h_prev is not None:
                emit_proj(h_prev, t - 1)
            h_prev = h_new
            h_cur = h_new
        emit_proj(h_prev, T - 1)

    nc.finalize()
    return nc


def _prep_inputs(latent, target, embed, W_ih, b_ih, W_hh, b_hh,
                 Wd0, bd0, Wd1, bd1, Wd2, bd2, Wp1, bp1, Wp2, bp2):
    f32 = np.float32
    latent = np.asarray(latent, dtype=f32)
    target = np.asarray(target)
    embed = np.asarray(embed, dtype=f32)
    W_ih = np.asarray(W_ih, dtype=f32)
    b_ih = np.asarray(b_ih, dtype=f32)
    W_hh = np.asarray(W_hh, dtype=f32)
    b_hh = np.asarray(b_hh, dtype=f32)

    # one-hot tokens (teacher forcing shift), time-major, plus a const-1 row
    tokens = np.concatenate(
        [np.zeros((B, 1), dtype=np.int64), np.asarray(target[:, :-1], dtype=np.int64)],
        axis=1,
    )  # [B, T]
    ohf = np.zeros((T, A + 1, B), dtype=BF16)
    tok_tm = tokens.T  # [T, B]
    for a in range(A):
        ohf[:, a, :] = (tok_tm == a)
    ohf[:, A, :] = 1.0

    giv = embed @ W_ih.T  # [A, 3H]
    brow = np.empty((G3,), dtype=f32)
    brow[: 2 * H] = (b_ih + b_hh)[: 2 * H]
    brow[2 * H:] = b_ih[2 * H:]
    giv_aug = np.concatenate([giv, brow[None, :]], axis=0).astype(BF16)  # [33, 768]

    whhT = np.ascontiguousarray(W_hh.T)  # [H, 3H]
    whh_l = np.ascontiguousarray(
        whhT.reshape(2, 128, G3).transpose(1, 0, 2)
    ).astype(BF16)  # [128, 2, 768]

    wd0_l = np.ascontiguousarray(np.asarray(Wd0, dtype=f32))          # [128, 256]
    wd1_l = np.ascontiguousarray(
        np.asarray(Wd1, dtype=f32).reshape(2, 128, H).transpose(1, 0, 2))
    wd2_l = np.ascontiguousarray(
        np.asarray(Wd2, dtype=f32).reshape(2, 128, H).transpose(1, 0, 2))
    wp1_l = np.ascontiguousarray(
        np.asarray(Wp1, dtype=f32).reshape(2, 128, A).transpose(1, 0, 2)).astype(BF16)
    wp2_l = np.ascontiguousarray(
        np.tile(np.asarray(Wp2, dtype=f32), (4, 1))).astype(BF16)  # [128, 32]

    bias_pack = np.zeros((128, 9), dtype=f32)
    bias_pack[:, 0] = b_hh[2 * H: 2 * H + 128]
    bias_pack[:, 1] = b_hh[2 * H + 128:]
    bias_pack[:, 2] = np.tile(np.asarray(bp1, dtype=f32), 4)
    bias_pack[:, 3] = np.asarray(bd0, dtype=f32)[:128]
    bias_pack[:, 4] = np.asarray(bd0, dtype=f32)[128:]
    bias_pack[:, 5] = np.asarray(bd1, dtype=f32)[:128]
    bias_pack[:, 6] = np.asarray(bd1, dtype=f32)[128:]
    bias_pack[:, 7] = np.asarray(bd2, dtype=f32)[:128]
    bias_pack[:, 8] = np.asarray(bd2, dtype=f32)[128:]

    bp2b = np.ascontiguousarray(
        np.tile(np.asarray(bp2, dtype=f32), 4)[None, :])  # [1, 128]

    # n-gate input table for the GPSIMD gather: giv_n.T + b_ih_n, [128, 2, A]
    givT_n = giv.T[2 * H:] + b_ih[2 * H:, None]          # [256, 32]
    gtab = np.ascontiguousarray(
        givT_n.reshape(2, 128, A).transpose(1, 0, 2)).astype(BF16)

    latT = np.ascontiguousarray(latent.T)  # [128, B]

    shared = dict(whh=whh_l, giv=giv_aug, wd0=wd0_l, wd1=wd1_l, wd2=wd2_l,
                  wp1=wp1_l, wp2=wp2_l, bias=bias_pack, bp2b=bp2b, gtab=gtab)
    in_maps = []
    for c in range(NCORES):
        bs = slice(c * BC, (c + 1) * BC)
        m = dict(shared)
        m["lat"] = np.ascontiguousarray(latT[:, bs])
        m["oh"] = np.ascontiguousarray(ohf[:, :, bs])
        # tokens wrapped for indirect_copy: index i lives at partition
        # (i%16), column (i//16), replicated across the 8 Q7 core groups
        tok_c = tokens[bs]                               # [1024, T]
        w = tok_c.reshape(64, 16, T).transpose(1, 2, 0)  # [16, T, 64]
        m["tokw"] = np.ascontiguousarray(
            np.tile(w, (8, 1, 1))).astype(np.uint16)     # [128, T, 64]
        in_maps.append(m)
    return in_maps


def kernel(**inputs):
    from concourse.bass_utils import run_bass_kernel_spmd

    if "nc" not in _CACHE:
        _CACHE["nc"] = _build()
    nc = _CACHE["nc"]

    in_maps = _prep_inputs(**inputs)
    res = run_bass_kernel_spmd(nc, in_maps, core_ids=list(range(NCORES)))
    outs = [r["out"] for r in res.results]
    return np.concatenate(outs, axis=0).astype(np.float32)
